# revision 23
# baseline (speedup 1.0000x reference)
"""Trainium2 Bass kernel for the AttentionBlock problem.

Reference semantics (shapes hardcoded):
    x [4, 256, 64, 64]; 1x1-conv weights q_w/k_w/v_w [256, 258] (+biases),
    fc_w [256, 256], fc_b [256].
    x0 = concat(x, pos) -> [B, 258, 4096]
    q/k/v = relu(W @ x0 + b)                    [B, 256, 4096]
    attn  = softmax_causal(q^T k)               [B, 4096, 4096]
    out   = x + relu(fc_w @ (attn @ v^T)^T + fc_b)

Distribution: 8 cores = 4 batches x 2 query-block roles. Each core
computes full k / v^T for its batch, q only for its 4 owned 512-wide
query blocks, and causal attention for those blocks. Causal work is
balanced by giving role 0 global blocks [0,3,4,7] and role 1 blocks
[1,2,5,6]; both roles run the identical SPMD program with per-slot
key-tile counts [8,16,24,32] (slightly padded), with per-core mask
data zeroing padded/non-causal entries.

DMA strategy (the prior bottleneck: everything on one HWDGE queue in
1KB packets ran at ~49 GB/s and paced the whole kernel): transfers
are split across BOTH TRN2 hardware DGE queues (SP-issued and
Activation-issued), x streams as per-pair 4KB-contiguous-run DMAs,
and ~1.1MB of former input traffic is generated on-chip instead:
  - the 4 canonical band masks T0..T3 via GpSimd memset+affine_select
  - the ones[128,128] lhsT via GpSimd memset
  - kbase/qbase pos+bias maps via tiny [3,128]x[3,512] matmuls from
    the w3 (pos-weight+bias) factors and the resident pos rows; the
    per-block rank-1 bias tables are host-computed RELATIVE to the
    core's first owned block since the generated base inherits that
    block's px offset.
The output is written as bf16 (the +x residual is added on-device
first), halving output bytes; the host gather upcasts to f32.

Softmax is computed without max-subtraction (scores are ~26+-5, far
from fp32 overflow): p = exp(s) * mask, normalized by a replicated
ones-matmul denominator (the [128,128] ones lhsT yields the column
sums broadcast across all partitions). Per-key-tile exps are summed
as bf16 quad sums + f32 running total, so only ONE ones matmul per
slot runs on the PE; all non-final quad sums run on the otherwise
idle GpSimd engine to unload VectorE.

Precision split: q/k/v projections contract bf16 copies of x (the
dominant cheap approximation; rel err ~7e-3 vs the 2e-2 gate), but
k/q are STORED as f32r so the q^T k score matmuls run at full f32r
precision (exp amplifies absolute score error, so the score matmul
itself must not round). attn-weight/v/fc arithmetic is bf16.

The k/q pos+bias contribution is NOT a third matmul: over a 512-wide
block it equals a fixed base map plus a per-block rank-1 term, so a
single fused VectorE scalar_tensor_tensor (psum + bias[128,1] +
base tile) replaces 24 matmuls. The first two pairs' k/q relus run
on VectorE (ScalarE's stream head is DMA issues), later ones on
ScalarE.

The fc stage runs on the UNNORMALIZED attention output (fc(po)/den
== fc(po/den)), letting the reciprocal overlap the fc matmuls; the
last slot runs its post-fc chain in half-width chunks so
mul/relu/add/DMA pipeline down the drain. Emission interleaves
phases two pairs ahead (pair0, q0, pair1, q1, slot0, pair2, ...) so
the PE always has runway while later transfers land. v relus run on
VectorE to keep ScalarE (exp-bound) off the phase-A critical path.
"""

import numpy as np

B = 4
C = 256
S = 64
N = S * S            # 4096
K = 256              # q/k/v channels
NBLK = 512           # query block width
NSLOT = 4            # owned query blocks per core
M_S = (8, 16, 24, 32)  # key-tile count per slot (128-wide key tiles)
BLOCKS = ((0, 3, 4, 7), (1, 2, 5, 6))  # role -> global block ids

_PROGRAM = None


def _build_program():
    import concourse.bacc as bacc
    import concourse.mybir as mybir
    import concourse.tile as tile

    F32 = mybir.dt.float32
    F32R = mybir.dt.float32r
    BF16 = mybir.dt.bfloat16
    Act = mybir.ActivationFunctionType
    Alu = mybir.AluOpType

    nc = bacc.Bacc("TRN2", target_bir_lowering=False, debug=False)

    # channel-half-merged inputs: [...][h][...] picks channel half h.
    # All 3-partition tensors (pos rows + w3 factors + coefficients) ride
    # ONE [3, 4878] tensor = 3 big packets: tiny multi-packet tensors
    # crawl (~300ns/packet) on the DGE queues regardless of queue.
    xb_d = nc.dram_tensor("xb", [128, 8, 2, NBLK], BF16,
                          kind="ExternalInput")
    s3_d = nc.dram_tensor("s3", [3, N + 3 * K + 14], BF16,
                          kind="ExternalInput")
    wq_d = nc.dram_tensor("wq", [128, 2, K], BF16, kind="ExternalInput")
    wk_d = nc.dram_tensor("wk", [128, 2, K], BF16, kind="ExternalInput")
    wv_d = nc.dram_tensor("wv", [128, 2, K], BF16, kind="ExternalInput")
    fcw_d = nc.dram_tensor("fcw", [128, 2, C], BF16, kind="ExternalInput")
    fcb_d = nc.dram_tensor("fcb", [128, 2], F32, kind="ExternalInput")
    out_d = nc.dram_tensor("out", [C, NSLOT * NBLK], BF16,
                           kind="ExternalOutput")

    with tile.TileContext(nc) as tc:
        with (
            tc.tile_pool(name="wts", bufs=1) as wts,
            tc.tile_pool(name="x0_p", bufs=4) as x0_p,
            tc.tile_pool(name="kqv_p", bufs=1) as kqv_p,
            tc.tile_pool(name="ex_p", bufs=9) as ex_p,
            tc.tile_pool(name="ds_p", bufs=3) as ds_p,
            tc.tile_pool(name="tot_p", bufs=2) as tot_p,
            tc.tile_pool(name="o_p", bufs=4) as o_p,
            tc.tile_pool(name="rb_p", bufs=2) as rb_p,
            tc.tile_pool(name="tr_p", bufs=2) as tr_p,
            tc.tile_pool(name="ps_sc", bufs=5, space="PSUM") as ps_sc,
            tc.tile_pool(name="ps_out", bufs=1, space="PSUM") as ps_out,
            tc.tile_pool(name="ps_mx", bufs=1, space="PSUM") as ps_mx,
        ):
            def wtile(dram, shape, dt, tag, eng=None):
                t = wts.tile(shape, dt, tag=tag, name=tag)
                (eng or nc.sync).dma_start(t[:], dram[:])
                return t

            k_sb = [[None] * 8 for _ in range(2)]
            vT_sb = [None] * 32
            q_sb = [[None] * NSLOT for _ in range(2)]

            def emit_pair_compute(nbp, xbp, early=False):
                # per-block k-then-v interleave so a late-arriving second
                # block never stalls work available on the first; early
                # pairs' k relus run on VectorE (ScalarE's stream head is
                # DMA issue instructions which would delay them)
                relu_eng = nc.vector if early else nc.scalar
                for li in range(2):
                    nb = 2 * nbp + li
                    for kt in range(2):
                        kts = slice(128 * kt, 128 * (kt + 1))
                        pk = ps_sc.tile([128, NBLK], F32, tag="sc",
                                        name=f"pk{kt}_{nb}")
                        nc.tensor.matmul(pk[:], wk_t[:, 0, kts],
                                         xbp[:, li, 0, :],
                                         start=True, stop=False)
                        nc.tensor.matmul(pk[:], wk_t[:, 1, kts],
                                         xbp[:, li, 1, :],
                                         start=False, stop=True)
                        # pos+bias contribution: rank-1 block bias + base map
                        km = tr_p.tile([128, NBLK], F32, tag="ktmp",
                                       name=f"km{kt}_{nb}")
                        nc.vector.scalar_tensor_tensor(
                            km[:], pk[:], kbias_t[:, 8 * kt + nb:8 * kt + nb + 1],
                            kbase_t[:, kt, :], Alu.add, Alu.add)
                        kt_sb = kqv_p.tile([128, NBLK], F32R,
                                           tag=f"k{kt}_{nb}",
                                           name=f"k{kt}_{nb}")
                        if early:
                            relu_eng.tensor_scalar_max(kt_sb[:], km[:], 0.0)
                        else:
                            relu_eng.activation(kt_sb[:], km[:], Act.Relu)
                        k_sb[kt][nb] = kt_sb
                    for sub in range(4):
                        i = 4 * nb + sub
                        ss = slice(128 * sub, 128 * (sub + 1))
                        p_ss = slice(NBLK * nb + 128 * sub,
                                     NBLK * nb + 128 * (sub + 1))
                        pv = ps_sc.tile([128, NBLK], F32, tag="sc",
                                        name=f"pv{i}")
                        nc.tensor.matmul(pv[:, :K], xbp[:, li, 0, ss],
                                         wv_t[:, 0, :],
                                         start=True, stop=False)
                        nc.tensor.matmul(pv[:, :K], xbp[:, li, 1, ss],
                                         wv_t[:, 1, :],
                                         start=False, stop=False)
                        nc.tensor.matmul(pv[:, :K], p3b_t[:, p_ss], wv3_t[:],
                                         start=False, stop=True)
                        vt_sb = kqv_p.tile([128, K], BF16, tag=f"v{i}",
                                           name=f"v{i}")
                        nc.vector.tensor_scalar_max(vt_sb[:], pv[:, :K], 0.0)
                        vT_sb[i] = vt_sb

            def emit_q(s, xqs, early=False):
                relu_eng = nc.vector if early else nc.scalar
                for kt in range(2):
                    kts = slice(128 * kt, 128 * (kt + 1))
                    pq = ps_sc.tile([128, NBLK], F32, tag="sc",
                                    name=f"pq{kt}_{s}")
                    nc.tensor.matmul(pq[:], wq_t[:, 0, kts], xqs[:, 0, :],
                                     start=True, stop=False)
                    nc.tensor.matmul(pq[:], wq_t[:, 1, kts], xqs[:, 1, :],
                                     start=False, stop=True)
                    qm = tr_p.tile([128, NBLK], F32, tag="ktmp",
                                   name=f"qm{kt}_{s}")
                    nc.vector.scalar_tensor_tensor(
                        qm[:], pq[:], qbias_t[:, 4 * kt + s:4 * kt + s + 1],
                        qbase_t[:, kt, :], Alu.add, Alu.add)
                    qt = kqv_p.tile([128, NBLK], F32R, tag=f"q{kt}_{s}",
                                    name=f"q{kt}_{s}")
                    if early:
                        relu_eng.tensor_scalar_max(qt[:], qm[:], 0.0)
                    else:
                        relu_eng.activation(qt[:], qm[:], Act.Relu)
                    q_sb[kt][s] = qt

            def finalize_slot(s, po, pd, xqs):
                """fc on unnormalized attention output, then normalize:
                fc(po/den) == fc(po)/den, so the reciprocal (VectorE)
                overlaps the fc matmuls instead of serializing them. The
                last slot runs the post-fc chain in half-width chunks so
                mul/relu/add/DMA pipeline down the drain."""
                nch = 2 if s == NSLOT - 1 else 1
                cw = NBLK // nch
                # po -> SBUF casts split across ScalarE and VectorE so
                # neither queue's backlog gates the fc weight loads
                o_sb = []
                for vt in range(2):
                    ot = o_p.tile([128, NBLK], BF16, tag="o",
                                  name=f"o{vt}_{s}")
                    if vt == 0:
                        nc.scalar.activation(ot[:], po[vt][:], Act.Copy)
                    else:
                        nc.vector.tensor_copy(ot[:], po[vt][:])
                    o_sb.append(ot)
                rb_sb = rb_p.tile([128, NBLK], F32, tag="rb", name=f"rb{s}")
                nc.vector.reciprocal_approx_fast(rb_sb[:], pd[:])
                for ot in range(2):
                    # the last slot's fc pairs use the freed score banks
                    # so they run without ps_mx bank serialization
                    pool = ps_sc if s == NSLOT - 1 else ps_mx
                    pfc = pool.tile([128, NBLK], F32,
                                    tag="sc" if s == NSLOT - 1 else "mx",
                                    name=f"pfc{ot}_{s}")
                    for vt in range(2):
                        nc.tensor.matmul(
                            pfc[:], fcw_t[:, vt, 128 * ot:128 * (ot + 1)],
                            o_sb[vt][:], start=(vt == 0), stop=(vt == 1))
                    u_sb = tr_p.tile([128, NBLK], F32, tag=f"t{ot}",
                                     name=f"u{ot}_{s}")
                    t_sb = tr_p.tile([128, NBLK], F32, tag=f"v{ot}",
                                     name=f"t{ot}_{s}")
                    r_sb = tr_p.tile([128, NBLK], BF16, tag=f"r{ot}",
                                     name=f"r{ot}_{s}")
                    # the last slot's second-half drain rides the other
                    # HWDGE queue so the two output streams overlap (no
                    # ScalarE-issued DMAs elsewhere in finalize: they
                    # would stall the next slot's exp stream)
                    dma_eng = (nc.scalar if s == NSLOT - 1 and ot == 1
                               else nc.sync)
                    for c in range(nch):
                        cs = slice(cw * c, cw * (c + 1))
                        nc.vector.tensor_mul(u_sb[:, cs], pfc[:, cs],
                                             rb_sb[:, cs])
                        nc.scalar.activation(t_sb[:, cs], u_sb[:, cs],
                                             Act.Relu,
                                             bias=fcb_t[:, ot:ot + 1])
                        nc.vector.tensor_add(r_sb[:, cs], t_sb[:, cs],
                                             xqs[:, ot, cs])
                        dma_eng.dma_start(
                            out_d[128 * ot:128 * (ot + 1),
                                  NBLK * s + cw * c:NBLK * s + cw * (c + 1)],
                            r_sb[:, cs])

            def emit_slot(s, fin):
                M = M_S[s]
                po = [ps_out.tile([128, NBLK], F32, tag=f"o{vt}",
                                  name=f"po{vt}_{s}") for vt in range(2)]
                ex_tiles = [None] * M
                tot = [None, None]

                def emit_scores(i):
                    # scores^T tile [128 keys, 512 queries]
                    psc = ps_sc.tile([128, NBLK], F32, tag="sc",
                                     name=f"psc{s}_{i}")
                    for kt in range(2):
                        nc.tensor.matmul(
                            psc[:],
                            k_sb[kt][i // 4][:, 128 * (i % 4):128 * (i % 4 + 1)],
                            q_sb[kt][s][:], start=(kt == 0), stop=(kt == 1))
                    ex = ex_p.tile([128, NBLK], BF16, tag="ex",
                                   name=f"ex{s}_{i}")
                    nc.scalar.activation(ex[:], psc[:], Act.Exp)
                    if i >= M - 8:
                        t4 = i - (M - 8)
                        if t4 < 4:
                            nc.vector.tensor_mul(
                                ex[:], ex[:], msk_t[:, t4, :])
                        else:
                            nc.vector.tensor_scalar_mul(
                                ex[:], ex[:], ct_t[:, s % 2:s % 2 + 1])
                    ex_tiles[i] = ex

                def consume_quad(j):
                    for jj in range(j, j + 4):
                        e = ex_tiles[jj]
                        for vt in range(2):
                            nc.tensor.matmul(
                                po[vt][:],
                                vT_sb[jj][:, 128 * vt:128 * (vt + 1)],
                                e[:], start=(jj == 0), stop=(jj == M - 1))
                    # bf16 quad sum, then f32 running total on VectorE; the
                    # last quad stays separate (accumulated by a second
                    # ones-matmul) so the tail does not wait on the tree
                    last = (j == M - 4)
                    da = ds_p.tile([128, NBLK], BF16, tag="ds",
                                   name=f"da{s}_{j}")
                    nc.vector.tensor_add(da[:], ex_tiles[j][:],
                                         ex_tiles[j + 1][:])
                    db = ds_p.tile([128, NBLK], BF16, tag="ds",
                                   name=f"db{s}_{j}")
                    nc.vector.tensor_add(db[:], ex_tiles[j + 2][:],
                                         ex_tiles[j + 3][:])
                    t = tot_p.tile([128, NBLK], F32R,
                                   tag="dsl" if last else "tot",
                                   name=f"tot{s}_{j}")
                    if last or tot[0] is None:
                        nc.vector.tensor_add(t[:], da[:], db[:])
                    else:
                        dsum = ds_p.tile([128, NBLK], BF16, tag="ds",
                                         name=f"ds{s}_{j}")
                        nc.vector.tensor_add(dsum[:], da[:], db[:])
                        nc.vector.tensor_add(t[:], tot[0][:], dsum[:])
                    if last:
                        tot[1] = t
                    else:
                        tot[0] = t
                    for jj in range(j, j + 4):
                        ex_tiles[jj] = None

                # 4-tile score batches between bf16 consume batches; the
                # previous slot's finalize is deferred to after the second
                # score batch so VectorE has runway for recip+muls
                for ib in range(0, M, 4):
                    for i in range(ib, ib + 4):
                        emit_scores(i)
                    if ib == 4 and fin is not None:
                        finalize_slot(*fin)
                    if ib >= 4:
                        consume_quad(ib - 4)
                # denominator tree-part matmul runs while the last
                # quad's exps cook; only the stop-matmul trails it
                pd = ps_mx.tile([128, NBLK], F32, tag="mx", name=f"pd{s}")
                nc.tensor.matmul(pd[:], ones_b[:], tot[0][:],
                                 start=True, stop=False)
                consume_quad(M - 4)
                nc.tensor.matmul(pd[:], ones_b[:], tot[1][:],
                                 start=False, stop=True)
                return po, pd

            # ---------------- emission schedule ----------------
            # Transfers split across both HWDGE queues in consumer order,
            # every stream leading with dense packets. SP queue: packed
            # small tensor, wk, blk0, wv, blk1, wq, fcw, then outputs.
            # Activation queue: pairs 1-3, fcb, slot3's second drain.
            s3_t = wtile(s3_d, [3, N + 3 * K + 14], BF16, "s3", nc.sync)
            p3b_t = s3_t[:, 0:N]
            w3k_t = s3_t[:, N:N + K]
            w3q_t = s3_t[:, N + K:N + 2 * K]
            wv3_t = s3_t[:, N + 2 * K:N + 3 * K]
            coef_t = s3_t[:, N + 3 * K:N + 3 * K + 14]
            wk_t = wtile(wk_d, [128, 2, K], BF16, "wk", nc.sync)
            xbp0 = x0_p.tile([128, 2, 2, NBLK], BF16, tag="xb0", name="xb_p0")
            nc.sync.dma_start(xbp0[:, 0], xb_d[:, 0])
            wv_t = wtile(wv_d, [128, 2, K], BF16, "wv", nc.sync)
            nc.sync.dma_start(xbp0[:, 1], xb_d[:, 1])
            wq_t = wtile(wq_d, [128, 2, K], BF16, "wq", nc.sync)
            fcw_t = wtile(fcw_d, [128, 2, C], BF16, "fcw", nc.sync)
            xbp1 = x0_p.tile([128, 2, 2, NBLK], BF16, tag="xb1", name="xb_p1")
            nc.scalar.dma_start(xbp1[:], xb_d[:, 2:4])
            xbp2 = x0_p.tile([128, 2, 2, NBLK], BF16, tag="xb2", name="xb_p2")
            nc.scalar.dma_start(xbp2[:], xb_d[:, 4:6])
            xbp3 = x0_p.tile([128, 2, 2, NBLK], BF16, tag="xb3", name="xb_p3")
            nc.scalar.dma_start(xbp3[:], xb_d[:, 6:8])
            fcb_t = wtile(fcb_d, [128, 2], F32, "fcb", nc.scalar)
            pair_t = [xbp0, xbp1, xbp2, xbp3]

            # on-chip mask/ones generation: a GpSimd iota (n - m) runs in
            # the dead time before the DMA rings come up, then VectorE
            # comparisons build the 4 band masks and the ones lhsT
            I16 = mybir.dt.int16
            iota_t = wts.tile([128, NBLK], I16, tag="iota", name="iota")
            nc.gpsimd.iota(iota_t[:], pattern=[[1, NBLK]], base=0,
                           channel_multiplier=-1)
            msk_t = wts.tile([128, 4, NBLK], BF16, tag="mk", name="msk")
            for r in range(4):
                # T_r[m, n] = (n - m >= 128r)
                nc.vector.tensor_scalar(msk_t[:, r, :], iota_t[:],
                                        128 * r, None, op0=Alu.is_ge)
            ones_b = wts.tile([128, 128], F32R, tag="ones_b", name="ones_b")
            nc.vector.tensor_scalar(ones_b[:], iota_t[:, :128],
                                    -32000, None, op0=Alu.is_ge)

            # bias tables / ct flags via tiny matmuls: their columns are
            # scalars times w_px (resp. ones), so a [3, n] coefficient
            # rhs against the w3 / pos-row lhsT reproduces them without
            # 128-partition tiny-packet DMAs
            kbias_t = wts.tile([128, 16], F32, tag="kbias", name="kbias")
            qbias_t = wts.tile([128, 8], F32, tag="qbias", name="qbias")
            ct_t = wts.tile([128, 2], F32, tag="ct", name="ct")
            pbias = ps_sc.tile([128, NBLK], F32, tag="sc", name="pbias")
            for kt in range(2):
                kts = slice(128 * kt, 128 * (kt + 1))
                nc.tensor.matmul(pbias[:, 8 * kt:8 * (kt + 1)],
                                 w3k_t[:, kts], coef_t[:, 0:8],
                                 start=True, stop=True)
                nc.tensor.matmul(pbias[:, 16 + 4 * kt:16 + 4 * (kt + 1)],
                                 w3q_t[:, kts], coef_t[:, 8:12],
                                 start=True, stop=True)
            nc.tensor.matmul(pbias[:, 24:26], p3b_t[:, 0:128],
                             coef_t[:, 12:14], start=True, stop=True)
            nc.vector.tensor_copy(kbias_t[:], pbias[:, 0:16])
            nc.vector.tensor_copy(qbias_t[:], pbias[:, 16:24])
            nc.vector.tensor_copy(ct_t[:], pbias[:, 24:26])

            # kbase/qbase pos+bias maps via [3,128]x[3,512] matmuls off
            # the core's first owned block's pos rows (bias tables are
            # host-shifted to be relative to that block)
            kbase_t = wts.tile([128, 2, NBLK], BF16, tag="kbase",
                               name="kbase")
            qbase_t = wts.tile([128, 2, NBLK], BF16, tag="qbase",
                               name="qbase")
            for dst, w3t in ((kbase_t, w3k_t), (qbase_t, w3q_t)):
                for h in range(2):
                    pb = ps_sc.tile([128, NBLK], F32, tag="sc",
                                    name=f"pbase{h}")
                    nc.tensor.matmul(pb[:], w3t[:, 128 * h:128 * (h + 1)],
                                     p3b_t[:, 0:NBLK], start=True, stop=True)
                    nc.vector.tensor_copy(dst[:, h, :], pb[:])

            xq_t = [pair_t[s][:, 0] for s in range(NSLOT)]

            emit_pair_compute(0, pair_t[0], early=True)
            emit_q(0, xq_t[0], early=True)
            emit_pair_compute(1, pair_t[1], early=True)
            emit_q(1, xq_t[1], early=True)

            pending = None
            for s in range(NSLOT):
                po, pd = emit_slot(s, pending)
                pending = (s, po, pd, xq_t[s])
                if s + 2 < NSLOT:
                    emit_pair_compute(s + 2, pair_t[s + 2])
                    emit_q(s + 2, xq_t[s + 2])
            finalize_slot(*pending)

    nc.compile()
    return nc


def _host_prep(x, q_w, q_b, k_w, k_b, v_w, v_b, fc_w, fc_b):
    """Build the per-core input maps."""
    import ml_dtypes
    f32 = np.float32
    bf16 = ml_dtypes.bfloat16
    n = np.arange(N)
    px = ((n // S) / S).astype(f32)
    py = ((n % S) / S).astype(f32)
    pos3 = np.stack([px, py, np.ones(N, f32)])   # [3, N] (incl bias channel)

    def merge_h(a):  # [256, M] -> [128, 2, M]
        return np.ascontiguousarray(a.reshape(2, 128, -1).transpose(1, 0, 2))

    def w3(w, b):
        # rows 0..1 = pos weight rows, row 2 = bias
        return np.ascontiguousarray(
            np.concatenate([w.astype(f32).T[C:], b.astype(f32)[None, :]], 0))

    shared = {
        "wq": merge_h(q_w.astype(f32).T[:C]).astype(bf16),
        "wk": merge_h(k_w.astype(f32).T[:C]).astype(bf16),
        "wv": merge_h(v_w.astype(f32).T[:C]).astype(bf16),
        "fcw": merge_h(fc_w.astype(f32).T).astype(bf16),
        "fcb": np.ascontiguousarray(fc_b.astype(f32).reshape(2, 128).T),
    }
    w3k = w3(k_w, k_b)
    w3q = w3(q_w, q_b)
    wv3 = w3(v_w, v_b)

    mm = np.arange(128)[:, None]
    nn = np.arange(NBLK)[None, :]
    in_maps = []
    for c in range(8):
        b, r = c // 2, c % 2
        xb = x[b].reshape(C, N).astype(f32)
        # local block order: owned block first within each pair
        order = []
        for p in range(NSLOT):
            j = BLOCKS[r][p]
            order += [j, j ^ 1]
        cols = np.concatenate(
            [np.arange(NBLK * j, NBLK * (j + 1)) for j in order])
        # masks for the local key-tile permutation
        mr = np.zeros((NSLOT, 8, 128, NBLK), f32)
        for s in range(NSLOT):
            j = BLOCKS[r][s]
            for t in range(8):
                i = M_S[s] - 8 + t
                gb = order[i // 4]
                mr[s, t] = (128 * (4 * gb + i % 4) + mm <= 512 * j + nn)
        Tt = (nn - mm >= 128 * np.arange(4)[:, None, None]).astype(f32)
        cset = np.zeros(2, f32)
        for s in range(NSLOT):
            assert np.array_equal(mr[s], mr[s % 2]), (r, s)
        for st in range(2):
            assert np.array_equal(mr[st, :4], Tt), (r, st)
            cset[st] = mr[st, 4, 0, 0]
            assert (mr[st, 4:] == cset[st]).all(), (r, st)
        # coefficient rhs for the on-chip bias/ct synthesis matmuls:
        # row 0 scales w_px by the block offset (relative to the first
        # owned block j0, whose px offset the generated base map already
        # carries), row 2 drives the ct flags off the pos ones-row
        j0 = order[0]
        coef = np.zeros((3, 14), f32)
        coef[0, 0:8] = [(j - j0) / 8.0 for j in order]
        coef[0, 8:12] = [(j - j0) / 8.0 for j in BLOCKS[r]]
        coef[2, 12:14] = cset
        in_maps.append(dict(
            shared,
            xb=np.ascontiguousarray(
                merge_h(xb[:, cols]).reshape(128, 2, 8, NBLK)
                .transpose(0, 2, 1, 3)).astype(bf16),
            s3=np.ascontiguousarray(np.concatenate(
                [pos3[:, cols], w3k, w3q, wv3, coef], axis=1)).astype(bf16),
        ))
    return in_maps


def _gather(results):
    out = np.empty((B, C, N), np.float32)
    for c in range(8):
        b, r = c // 2, c % 2
        oc = results[c]["out"]
        for s, j in enumerate(BLOCKS[r]):
            out[b][:, NBLK * j:NBLK * (j + 1)] = (
                oc[:, NBLK * s:NBLK * (s + 1)].astype(np.float32))
    return out.reshape(B, C, S, S)


def run(trace=False, **inputs):
    from concourse import bass_utils
    global _PROGRAM
    if _PROGRAM is None:
        _PROGRAM = _build_program()
    in_maps = _host_prep(**inputs)
    res = bass_utils.run_bass_kernel_spmd(
        _PROGRAM, in_maps, list(range(8)), trace=trace)
    return _gather(res.results), res


def kernel(**inputs):
    out, _ = run(trace=False, **inputs)
    return out


# revision 27
# speedup vs baseline: 1.0318x; 1.0318x over previous
"""Trainium2 Bass kernel for the AttentionBlock problem.

Reference semantics (shapes hardcoded):
    x [4, 256, 64, 64]; 1x1-conv weights q_w/k_w/v_w [256, 258] (+biases),
    fc_w [256, 256], fc_b [256].
    x0 = concat(x, pos) -> [B, 258, 4096]
    q/k/v = relu(W @ x0 + b)                    [B, 256, 4096]
    attn  = softmax_causal(q^T k)               [B, 4096, 4096]
    out   = x + relu(fc_w @ (attn @ v^T)^T + fc_b)

Distribution: 8 cores = 4 batches x 2 query-block roles. Each core
computes full k / v^T for its batch, q only for its 4 owned 512-wide
query blocks, and causal attention for those blocks. Causal work is
balanced by giving role 0 global blocks [0,3,4,7] and role 1 blocks
[1,2,5,6]; both roles run the identical SPMD program with per-slot
key-tile counts [8,16,24,32] (slightly padded), with per-core mask
data zeroing padded/non-causal entries.

DMA strategy (the prior bottleneck: everything on one HWDGE queue in
1KB packets ran at ~49 GB/s and paced the whole kernel): transfers
are split across BOTH TRN2 hardware DGE queues (SP-issued and
Activation-issued), x streams as per-pair 4KB-contiguous-run DMAs,
and ~1.1MB of former input traffic is generated on-chip instead:
  - the 4 canonical band masks T0..T3 via GpSimd memset+affine_select
  - the ones[128,128] lhsT via GpSimd memset
  - kbase/qbase pos+bias maps via tiny [3,128]x[3,512] matmuls from
    the w3 (pos-weight+bias) factors and the resident pos rows; the
    per-block rank-1 bias tables are host-computed RELATIVE to the
    core's first owned block since the generated base inherits that
    block's px offset.
The output is written as bf16 (the +x residual is added on-device
first), halving output bytes; the host gather upcasts to f32.

Softmax is computed without max-subtraction (scores are ~26+-5, far
from fp32 overflow): p = exp(s) * mask, normalized by a replicated
ones-matmul denominator (the [128,128] ones lhsT yields the column
sums broadcast across all partitions). Per-key-tile exps are summed
as bf16 quad sums + f32 running total, so only ONE ones matmul per
slot runs on the PE; all non-final quad sums run on the otherwise
idle GpSimd engine to unload VectorE.

Precision split: q/k/v projections contract bf16 copies of x (the
dominant cheap approximation; rel err ~7e-3 vs the 2e-2 gate), but
k/q are STORED as f32r so the q^T k score matmuls run at full f32r
precision (exp amplifies absolute score error, so the score matmul
itself must not round). attn-weight/v/fc arithmetic is bf16.

The k/q pos+bias contribution is NOT a third matmul: over a 512-wide
block it equals a fixed base map plus a per-block rank-1 term, so a
single fused VectorE scalar_tensor_tensor (psum + bias[128,1] +
base tile) replaces 24 matmuls. The first two pairs' k/q relus run
on VectorE (ScalarE's stream head is DMA issues), later ones on
ScalarE.

The fc stage runs on the UNNORMALIZED attention output (fc(po)/den
== fc(po/den)), letting the reciprocal overlap the fc matmuls; the
last slot runs its post-fc chain in half-width chunks so
mul/relu/add/DMA pipeline down the drain. Emission interleaves
phases two pairs ahead (pair0, q0, pair1, q1, slot0, pair2, ...) so
the PE always has runway while later transfers land. v relus run on
VectorE to keep ScalarE (exp-bound) off the phase-A critical path.
"""

import numpy as np

B = 4
C = 256
S = 64
N = S * S            # 4096
K = 256              # q/k/v channels
NBLK = 512           # query block width
NSLOT = 4            # owned query blocks per core
M_S = (8, 16, 24, 32)  # key-tile count per slot (128-wide key tiles)
BLOCKS = ((0, 3, 4, 7), (1, 2, 5, 6))  # role -> global block ids

_PROGRAM = None


def _build_program():
    import concourse.bacc as bacc
    import concourse.mybir as mybir
    import concourse.tile as tile

    F32 = mybir.dt.float32
    F32R = mybir.dt.float32r
    BF16 = mybir.dt.bfloat16
    Act = mybir.ActivationFunctionType
    Alu = mybir.AluOpType

    nc = bacc.Bacc("TRN2", target_bir_lowering=False, debug=False)

    # channel-half-merged inputs: [...][h][...] picks channel half h.
    # All 3-partition tensors (pos rows + w3 factors + coefficients) ride
    # ONE [3, 4878] tensor = 3 big packets: tiny multi-packet tensors
    # crawl (~300ns/packet) on the DGE queues regardless of queue.
    xb_d = nc.dram_tensor("xb", [128, 8, 2, NBLK], BF16,
                          kind="ExternalInput")
    s3_d = nc.dram_tensor("s3", [3, N + 3 * K + 14], BF16,
                          kind="ExternalInput")
    wq_d = nc.dram_tensor("wq", [128, 2, K], BF16, kind="ExternalInput")
    wk_d = nc.dram_tensor("wk", [128, 2, K], BF16, kind="ExternalInput")
    wv_d = nc.dram_tensor("wv", [128, 2, K], BF16, kind="ExternalInput")
    fcw_d = nc.dram_tensor("fcw", [128, 2, C], BF16, kind="ExternalInput")
    fcb_d = nc.dram_tensor("fcb", [128, 2], F32, kind="ExternalInput")
    out_d = nc.dram_tensor("out", [C, NSLOT * NBLK], BF16,
                           kind="ExternalOutput")

    with tile.TileContext(nc) as tc:
        with (
            tc.tile_pool(name="wts", bufs=1) as wts,
            tc.tile_pool(name="x0_p", bufs=4) as x0_p,
            tc.tile_pool(name="kqv_p", bufs=1) as kqv_p,
            tc.tile_pool(name="ex_p", bufs=9) as ex_p,
            tc.tile_pool(name="ds_p", bufs=6) as ds_p,
            tc.tile_pool(name="tot_p", bufs=2) as tot_p,
            tc.tile_pool(name="o_p", bufs=4) as o_p,
            tc.tile_pool(name="rb_p", bufs=2) as rb_p,
            tc.tile_pool(name="tr_p", bufs=2) as tr_p,
            tc.tile_pool(name="ps_sc", bufs=5, space="PSUM") as ps_sc,
            tc.tile_pool(name="ps_out", bufs=1, space="PSUM") as ps_out,
            tc.tile_pool(name="ps_mx", bufs=1, space="PSUM") as ps_mx,
        ):
            def wtile(dram, shape, dt, tag, eng=None):
                t = wts.tile(shape, dt, tag=tag, name=tag)
                (eng or nc.sync).dma_start(t[:], dram[:])
                return t

            k_sb = [[None] * 8 for _ in range(2)]
            vT_sb = [None] * 32
            q_sb = [[None] * NSLOT for _ in range(2)]

            def emit_pair_compute(nbp, xbp):
                # per-block k-then-v interleave so a late-arriving second
                # block never stalls work available on the first
                for li in range(2):
                    nb = 2 * nbp + li
                    for kt in range(2):
                        kts = slice(128 * kt, 128 * (kt + 1))
                        pk = ps_sc.tile([128, NBLK], F32, tag="sc",
                                        name=f"pk{kt}_{nb}")
                        nc.tensor.matmul(pk[:], wk_t[:, 0, kts],
                                         xbp[:, li, 0, :],
                                         start=True, stop=False)
                        nc.tensor.matmul(pk[:], wk_t[:, 1, kts],
                                         xbp[:, li, 1, :],
                                         start=False, stop=True)
                        # pos+bias contribution: rank-1 block bias + base map
                        km = tr_p.tile([128, NBLK], F32, tag="ktmp",
                                       name=f"km{kt}_{nb}")
                        nc.vector.scalar_tensor_tensor(
                            km[:], pk[:], kbias_t[:, 8 * kt + nb:8 * kt + nb + 1],
                            kbase_t[:, kt, :], Alu.add, Alu.add)
                        kt_sb = kqv_p.tile([128, NBLK], F32R,
                                           tag=f"k{kt}_{nb}",
                                           name=f"k{kt}_{nb}")
                        nc.scalar.activation(kt_sb[:], km[:], Act.Relu)
                        k_sb[kt][nb] = kt_sb
                    for sub in range(4):
                        i = 4 * nb + sub
                        ss = slice(128 * sub, 128 * (sub + 1))
                        p_ss = slice(NBLK * nb + 128 * sub,
                                     NBLK * nb + 128 * (sub + 1))
                        pv = ps_sc.tile([128, NBLK], F32, tag="sc",
                                        name=f"pv{i}")
                        nc.tensor.matmul(pv[:, :K], xbp[:, li, 0, ss],
                                         wv_t[:, 0, :],
                                         start=True, stop=False)
                        nc.tensor.matmul(pv[:, :K], xbp[:, li, 1, ss],
                                         wv_t[:, 1, :],
                                         start=False, stop=False)
                        nc.tensor.matmul(pv[:, :K], p3b_t[:, p_ss], wv3_t[:],
                                         start=False, stop=True)
                        vt_sb = kqv_p.tile([128, K], BF16, tag=f"v{i}",
                                           name=f"v{i}")
                        nc.vector.tensor_scalar_max(vt_sb[:], pv[:, :K], 0.0)
                        vT_sb[i] = vt_sb

            def emit_q(s, xqs):
                for kt in range(2):
                    kts = slice(128 * kt, 128 * (kt + 1))
                    pq = ps_sc.tile([128, NBLK], F32, tag="sc",
                                    name=f"pq{kt}_{s}")
                    nc.tensor.matmul(pq[:], wq_t[:, 0, kts], xqs[:, 0, :],
                                     start=True, stop=False)
                    nc.tensor.matmul(pq[:], wq_t[:, 1, kts], xqs[:, 1, :],
                                     start=False, stop=True)
                    qm = tr_p.tile([128, NBLK], F32, tag="ktmp",
                                   name=f"qm{kt}_{s}")
                    nc.vector.scalar_tensor_tensor(
                        qm[:], pq[:], qbias_t[:, 4 * kt + s:4 * kt + s + 1],
                        qbase_t[:, kt, :], Alu.add, Alu.add)
                    qt = kqv_p.tile([128, NBLK], F32R, tag=f"q{kt}_{s}",
                                    name=f"q{kt}_{s}")
                    nc.scalar.activation(qt[:], qm[:], Act.Relu)
                    q_sb[kt][s] = qt

            def finalize_slot(s, po, pd, xqs):
                """fc on unnormalized attention output, then normalize:
                fc(po/den) == fc(po)/den, so the reciprocal (VectorE)
                overlaps the fc matmuls instead of serializing them. The
                last slot runs the post-fc chain in half-width chunks so
                mul/relu/add/DMA pipeline down the drain."""
                nch = 2 if s == NSLOT - 1 else 1
                cw = NBLK // nch
                # po -> SBUF casts split across ScalarE and VectorE so
                # neither queue's backlog gates the fc weight loads
                o_sb = []
                for vt in range(2):
                    ot = o_p.tile([128, NBLK], BF16, tag="o",
                                  name=f"o{vt}_{s}")
                    if vt == 0:
                        nc.scalar.activation(ot[:], po[vt][:], Act.Copy)
                    else:
                        nc.vector.tensor_copy(ot[:], po[vt][:])
                    o_sb.append(ot)
                rb_sb = rb_p.tile([128, NBLK], F32, tag="rb", name=f"rb{s}")
                nc.vector.reciprocal_approx_fast(rb_sb[:], pd[:])
                for ot in range(2):
                    # the last slot's fc pairs use the freed score banks
                    # so they run without ps_mx bank serialization
                    pool = ps_sc if s == NSLOT - 1 else ps_mx
                    pfc = pool.tile([128, NBLK], F32,
                                    tag="sc" if s == NSLOT - 1 else "mx",
                                    name=f"pfc{ot}_{s}")
                    for vt in range(2):
                        nc.tensor.matmul(
                            pfc[:], fcw_t[:, vt, 128 * ot:128 * (ot + 1)],
                            o_sb[vt][:], start=(vt == 0), stop=(vt == 1))
                    u_sb = tr_p.tile([128, NBLK], F32, tag=f"t{ot}",
                                     name=f"u{ot}_{s}")
                    t_sb = tr_p.tile([128, NBLK], F32, tag=f"v{ot}",
                                     name=f"t{ot}_{s}")
                    r_sb = tr_p.tile([128, NBLK], BF16, tag=f"r{ot}",
                                     name=f"r{ot}_{s}")
                    # the last slot's second-half drain rides the other
                    # HWDGE queue so the two output streams overlap (no
                    # ScalarE-issued DMAs elsewhere in finalize: they
                    # would stall the next slot's exp stream)
                    dma_eng = (nc.scalar if s == NSLOT - 1 and ot == 1
                               else nc.sync)
                    for c in range(nch):
                        cs = slice(cw * c, cw * (c + 1))
                        nc.vector.tensor_mul(u_sb[:, cs], pfc[:, cs],
                                             rb_sb[:, cs])
                        nc.scalar.activation(t_sb[:, cs], u_sb[:, cs],
                                             Act.Relu,
                                             bias=fcb_t[:, ot:ot + 1])
                        nc.vector.tensor_add(r_sb[:, cs], t_sb[:, cs],
                                             xqs[:, ot, cs])
                        dma_eng.dma_start(
                            out_d[128 * ot:128 * (ot + 1),
                                  NBLK * s + cw * c:NBLK * s + cw * (c + 1)],
                            r_sb[:, cs])

            def emit_slot(s, fin):
                M = M_S[s]
                po = [ps_out.tile([128, NBLK], F32, tag=f"o{vt}",
                                  name=f"po{vt}_{s}") for vt in range(2)]
                ex_tiles = [None] * M
                tot = [None, None]

                def emit_scores(i):
                    # scores^T tile [128 keys, 512 queries]
                    psc = ps_sc.tile([128, NBLK], F32, tag="sc",
                                     name=f"psc{s}_{i}")
                    for kt in range(2):
                        nc.tensor.matmul(
                            psc[:],
                            k_sb[kt][i // 4][:, 128 * (i % 4):128 * (i % 4 + 1)],
                            q_sb[kt][s][:], start=(kt == 0), stop=(kt == 1))
                    ex = ex_p.tile([128, NBLK], BF16, tag="ex",
                                   name=f"ex{s}_{i}")
                    # padded/role-inactive tiles zero for free through the
                    # exp's per-partition bias: exp(s - 10000) == 0
                    t4 = i - (M - 8)
                    if 4 <= t4 < 8:
                        nc.scalar.activation(ex[:], psc[:], Act.Exp,
                                             bias=ctb_t[:, s % 2:s % 2 + 1])
                    else:
                        nc.scalar.activation(ex[:], psc[:], Act.Exp)
                    if 0 <= t4 < 4:
                        nc.vector.tensor_mul(ex[:], ex[:], msk_t[:, t4, :])
                    ex_tiles[i] = ex

                def consume_quad(j):
                    for jj in range(j, j + 4):
                        e = ex_tiles[jj]
                        for vt in range(2):
                            nc.tensor.matmul(
                                po[vt][:],
                                vT_sb[jj][:, 128 * vt:128 * (vt + 1)],
                                e[:], start=(jj == 0), stop=(jj == M - 1))
                    # bf16 quad sum, then f32 running total; the bulk of
                    # the tree runs on the otherwise idle GpSimd (slow per
                    # op but with multi-microsecond slack), only the last
                    # quad — which trails into the denominator's stop-
                    # matmul — stays latency-critical on VectorE
                    last = (j == M - 4)
                    sum_eng = nc.vector if last else nc.gpsimd
                    da = ds_p.tile([128, NBLK], BF16, tag="ds",
                                   name=f"da{s}_{j}")
                    sum_eng.tensor_add(da[:], ex_tiles[j][:],
                                       ex_tiles[j + 1][:])
                    db = ds_p.tile([128, NBLK], BF16, tag="ds",
                                   name=f"db{s}_{j}")
                    sum_eng.tensor_add(db[:], ex_tiles[j + 2][:],
                                       ex_tiles[j + 3][:])
                    t = tot_p.tile([128, NBLK], F32R,
                                   tag="dsl" if last else "tot",
                                   name=f"tot{s}_{j}")
                    if last or tot[0] is None:
                        sum_eng.tensor_add(t[:], da[:], db[:])
                    else:
                        dsum = ds_p.tile([128, NBLK], BF16, tag="ds",
                                         name=f"ds{s}_{j}")
                        nc.gpsimd.tensor_add(dsum[:], da[:], db[:])
                        nc.vector.tensor_add(t[:], tot[0][:], dsum[:])
                    if last:
                        tot[1] = t
                    else:
                        tot[0] = t
                    for jj in range(j, j + 4):
                        ex_tiles[jj] = None

                # 4-tile score batches between bf16 consume batches; the
                # previous slot's finalize is deferred to after the second
                # score batch so VectorE has runway for recip+muls
                for ib in range(0, M, 4):
                    for i in range(ib, ib + 4):
                        emit_scores(i)
                    if ib == 4 and fin is not None:
                        finalize_slot(*fin)
                    if ib >= 4:
                        consume_quad(ib - 4)
                # denominator tree-part matmul runs while the last
                # quad's exps cook; only the stop-matmul trails it
                pd = ps_mx.tile([128, NBLK], F32, tag="mx", name=f"pd{s}")
                nc.tensor.matmul(pd[:], ones_b[:], tot[0][:],
                                 start=True, stop=False)
                consume_quad(M - 4)
                nc.tensor.matmul(pd[:], ones_b[:], tot[1][:],
                                 start=False, stop=True)
                return po, pd

            # ---------------- emission schedule ----------------
            # Transfers split across both HWDGE queues in consumer order,
            # every stream leading with dense packets. SP queue: packed
            # small tensor, wk, blk0, wv, blk1, wq, fcw, then outputs.
            # Activation queue: pairs 1-3, fcb, slot3's second drain.
            s3_t = wtile(s3_d, [3, N + 3 * K + 14], BF16, "s3", nc.sync)
            p3b_t = s3_t[:, 0:N]
            w3k_t = s3_t[:, N:N + K]
            w3q_t = s3_t[:, N + K:N + 2 * K]
            wv3_t = s3_t[:, N + 2 * K:N + 3 * K]
            coef_t = s3_t[:, N + 3 * K:N + 3 * K + 14]
            wk_t = wtile(wk_d, [128, 2, K], BF16, "wk", nc.sync)
            xbp0 = x0_p.tile([128, 2, 2, NBLK], BF16, tag="xb0", name="xb_p0")
            nc.sync.dma_start(xbp0[:, 0], xb_d[:, 0])
            wv_t = wtile(wv_d, [128, 2, K], BF16, "wv", nc.sync)
            nc.sync.dma_start(xbp0[:, 1], xb_d[:, 1])
            wq_t = wtile(wq_d, [128, 2, K], BF16, "wq", nc.sync)
            fcw_t = wtile(fcw_d, [128, 2, C], BF16, "fcw", nc.sync)
            xbp1 = x0_p.tile([128, 2, 2, NBLK], BF16, tag="xb1", name="xb_p1")
            nc.scalar.dma_start(xbp1[:], xb_d[:, 2:4])
            xbp2 = x0_p.tile([128, 2, 2, NBLK], BF16, tag="xb2", name="xb_p2")
            nc.scalar.dma_start(xbp2[:], xb_d[:, 4:6])
            xbp3 = x0_p.tile([128, 2, 2, NBLK], BF16, tag="xb3", name="xb_p3")
            nc.scalar.dma_start(xbp3[:], xb_d[:, 6:8])
            fcb_t = wtile(fcb_d, [128, 2], F32, "fcb", nc.scalar)
            pair_t = [xbp0, xbp1, xbp2, xbp3]

            # on-chip mask/ones generation: a GpSimd iota (n - m) runs in
            # the dead time before the DMA rings come up, then VectorE
            # comparisons build the 4 band masks and the ones lhsT
            I16 = mybir.dt.int16
            iota_t = wts.tile([128, NBLK], I16, tag="iota", name="iota")
            nc.gpsimd.iota(iota_t[:], pattern=[[1, NBLK]], base=0,
                           channel_multiplier=-1)
            msk_t = wts.tile([128, 4, NBLK], BF16, tag="mk", name="msk")
            for r in range(4):
                # T_r[m, n] = (n - m >= 128r)
                nc.vector.tensor_scalar(msk_t[:, r, :], iota_t[:],
                                        128 * r, None, op0=Alu.is_ge)
            ones_b = wts.tile([128, 128], F32R, tag="ones_b", name="ones_b")
            nc.vector.tensor_scalar(ones_b[:], iota_t[:, :128],
                                    -32000, None, op0=Alu.is_ge)

            # bias tables / ct flags via tiny matmuls: their columns are
            # scalars times w_px (resp. ones), so a [3, n] coefficient
            # rhs against the w3 / pos-row lhsT reproduces them without
            # 128-partition tiny-packet DMAs
            kbias_t = wts.tile([128, 16], F32, tag="kbias", name="kbias")
            qbias_t = wts.tile([128, 8], F32, tag="qbias", name="qbias")
            ctb_t = wts.tile([128, 2], F32, tag="ctb", name="ctb")
            pbias = ps_sc.tile([128, NBLK], F32, tag="sc", name="pbias")
            for kt in range(2):
                kts = slice(128 * kt, 128 * (kt + 1))
                nc.tensor.matmul(pbias[:, 8 * kt:8 * (kt + 1)],
                                 w3k_t[:, kts], coef_t[:, 0:8],
                                 start=True, stop=True)
                nc.tensor.matmul(pbias[:, 16 + 4 * kt:16 + 4 * (kt + 1)],
                                 w3q_t[:, kts], coef_t[:, 8:12],
                                 start=True, stop=True)
            nc.tensor.matmul(pbias[:, 24:26], p3b_t[:, 0:128],
                             coef_t[:, 12:14], start=True, stop=True)
            nc.vector.tensor_copy(kbias_t[:], pbias[:, 0:16])
            nc.vector.tensor_copy(qbias_t[:], pbias[:, 16:24])
            nc.vector.tensor_copy(ctb_t[:], pbias[:, 24:26])

            # kbase/qbase pos+bias maps via [3,128]x[3,512] matmuls off
            # the core's first owned block's pos rows (bias tables are
            # host-shifted to be relative to that block)
            kbase_t = wts.tile([128, 2, NBLK], BF16, tag="kbase",
                               name="kbase")
            qbase_t = wts.tile([128, 2, NBLK], BF16, tag="qbase",
                               name="qbase")
            for dst, w3t in ((kbase_t, w3k_t), (qbase_t, w3q_t)):
                for h in range(2):
                    pb = ps_sc.tile([128, NBLK], F32, tag="sc",
                                    name=f"pbase{h}")
                    nc.tensor.matmul(pb[:], w3t[:, 128 * h:128 * (h + 1)],
                                     p3b_t[:, 0:NBLK], start=True, stop=True)
                    nc.vector.tensor_copy(dst[:, h, :], pb[:])

            xq_t = [pair_t[s][:, 0] for s in range(NSLOT)]

            emit_pair_compute(0, pair_t[0])
            emit_q(0, xq_t[0])
            emit_pair_compute(1, pair_t[1])
            emit_q(1, xq_t[1])

            pending = None
            for s in range(NSLOT):
                po, pd = emit_slot(s, pending)
                pending = (s, po, pd, xq_t[s])
                if s + 2 < NSLOT:
                    emit_pair_compute(s + 2, pair_t[s + 2])
                    emit_q(s + 2, xq_t[s + 2])
            finalize_slot(*pending)

    nc.compile()
    return nc


def _host_prep(x, q_w, q_b, k_w, k_b, v_w, v_b, fc_w, fc_b):
    """Build the per-core input maps."""
    import ml_dtypes
    f32 = np.float32
    bf16 = ml_dtypes.bfloat16
    n = np.arange(N)
    px = ((n // S) / S).astype(f32)
    py = ((n % S) / S).astype(f32)
    pos3 = np.stack([px, py, np.ones(N, f32)])   # [3, N] (incl bias channel)

    def merge_h(a):  # [256, M] -> [128, 2, M]
        return np.ascontiguousarray(a.reshape(2, 128, -1).transpose(1, 0, 2))

    def w3(w, b):
        # rows 0..1 = pos weight rows, row 2 = bias
        return np.ascontiguousarray(
            np.concatenate([w.astype(f32).T[C:], b.astype(f32)[None, :]], 0))

    shared = {
        "wq": merge_h(q_w.astype(f32).T[:C]).astype(bf16),
        "wk": merge_h(k_w.astype(f32).T[:C]).astype(bf16),
        "wv": merge_h(v_w.astype(f32).T[:C]).astype(bf16),
        "fcw": merge_h(fc_w.astype(f32).T).astype(bf16),
        "fcb": np.ascontiguousarray(fc_b.astype(f32).reshape(2, 128).T),
    }
    w3k = w3(k_w, k_b)
    w3q = w3(q_w, q_b)
    wv3 = w3(v_w, v_b)

    mm = np.arange(128)[:, None]
    nn = np.arange(NBLK)[None, :]
    in_maps = []
    for c in range(8):
        b, r = c // 2, c % 2
        xb = x[b].reshape(C, N).astype(f32)
        # local block order: owned block first within each pair
        order = []
        for p in range(NSLOT):
            j = BLOCKS[r][p]
            order += [j, j ^ 1]
        cols = np.concatenate(
            [np.arange(NBLK * j, NBLK * (j + 1)) for j in order])
        # masks for the local key-tile permutation
        mr = np.zeros((NSLOT, 8, 128, NBLK), f32)
        for s in range(NSLOT):
            j = BLOCKS[r][s]
            for t in range(8):
                i = M_S[s] - 8 + t
                gb = order[i // 4]
                mr[s, t] = (128 * (4 * gb + i % 4) + mm <= 512 * j + nn)
        Tt = (nn - mm >= 128 * np.arange(4)[:, None, None]).astype(f32)
        cset = np.zeros(2, f32)
        for s in range(NSLOT):
            assert np.array_equal(mr[s], mr[s % 2]), (r, s)
        for st in range(2):
            assert np.array_equal(mr[st, :4], Tt), (r, st)
            cset[st] = mr[st, 4, 0, 0]
            assert (mr[st, 4:] == cset[st]).all(), (r, st)
        # coefficient rhs for the on-chip bias/ct synthesis matmuls:
        # row 0 scales w_px by the block offset (relative to the first
        # owned block j0, whose px offset the generated base map already
        # carries), row 2 drives the ct flags off the pos ones-row
        j0 = order[0]
        coef = np.zeros((3, 14), f32)
        coef[0, 0:8] = [(j - j0) / 8.0 for j in order]
        coef[0, 8:12] = [(j - j0) / 8.0 for j in BLOCKS[r]]
        # exp-bias form of the padded-tile kill switch: exp(s - 10000)
        # underflows to exactly 0, exp(s - 0) is untouched
        coef[2, 12:14] = -10000.0 * (1.0 - cset)
        in_maps.append(dict(
            shared,
            xb=np.ascontiguousarray(
                merge_h(xb[:, cols]).reshape(128, 2, 8, NBLK)
                .transpose(0, 2, 1, 3)).astype(bf16),
            s3=np.ascontiguousarray(np.concatenate(
                [pos3[:, cols], w3k, w3q, wv3, coef], axis=1)).astype(bf16),
        ))
    return in_maps


def _gather(results):
    out = np.empty((B, C, N), np.float32)
    for c in range(8):
        b, r = c // 2, c % 2
        oc = results[c]["out"]
        for s, j in enumerate(BLOCKS[r]):
            out[b][:, NBLK * j:NBLK * (j + 1)] = (
                oc[:, NBLK * s:NBLK * (s + 1)].astype(np.float32))
    return out.reshape(B, C, S, S)


def run(trace=False, **inputs):
    from concourse import bass_utils
    global _PROGRAM
    if _PROGRAM is None:
        _PROGRAM = _build_program()
    in_maps = _host_prep(**inputs)
    res = bass_utils.run_bass_kernel_spmd(
        _PROGRAM, in_maps, list(range(8)), trace=trace)
    return _gather(res.results), res


def kernel(**inputs):
    out, _ = run(trace=False, **inputs)
    return out
